# revision 25
# baseline (speedup 1.0000x reference)
"""Cross-attention kernel for Trainium2 (8 NeuronCores, batch-parallel).

Reference computation (per batch element b):
    q = x @ Wq + bq            # [T, E]
    k = y @ Wk + bk            # [S, E]
    v = y @ Wv + bv            # [S, E]
    per head h (D=80): scores = q_h @ k_h.T / sqrt(D); A = softmax(scores)
    attn = concat_h(A @ v_h)   # [T, E]
    out = attn @ Wo + bo       # [T, E]

Sharding: batch (8) across the 8 cores, one batch element per core.

On-chip layout is feature-major (x and the output are transposed on the
host so every DMA is a contiguous row load/store and no on-chip
transposes are needed):
    xt  = x[b].T   [E, T]   -> X' tiles [128 c, 512 t] (bf16)
    q'  = Wq.T@X'  [E, T]   feature-major (e on partitions)
    k'  staged per head fragment with zero padding (80-dim heads vs
        128-partition tiles; matmul operand base partitions must be 0)
    U   = exp(scores) @ V computed unnormalized; the softmax denominator
        r8[h,t] = sum_s exp(scores) is accumulated with a ones-matmul
        (ecols) into one [H, TC] psum tile, inverted on DVE, and
        broadcast onto the 5 attn e'-tiles with 5 tiny sel-matmuls
        (K=8); normalization is one DVE multiply per tile fused with
        the PSUM->SBUF move (out in bf16 for the O-proj).
    out' = Wo.T @ attn' [E, T]; host transposes back.

HBM-resident operands (x, y, Wq, Wk, Wv, Wo) are bf16: same PE rate as
float32r at free-dim 512, half the DMA traffic, and full rate for the
free-dim-77 K-projection (f32r would take a 4x penalty there).
"""

import numpy as np

import concourse.bass as bass
import concourse.mybir as mybir
import concourse.tile as tile
from concourse import bacc
from concourse.bass_utils import run_bass_kernel_spmd

F32 = mybir.dt.float32
F32R = mybir.dt.float32r
BF16 = mybir.dt.bfloat16
FP8 = mybir.dt.float8e4
DR = mybir.MatmulPerfMode.DoubleRow
AF = mybir.ActivationFunctionType

B, T, E, CR, H, D, S = 8, 4096, 640, 768, 8, 80, 77
CHUNKS = ([(0, 256), (256, 256)]
          + [(512 + 512 * i, 512) for i in range(6)]
          + [(3584, 256), (3840, 256)])
TC = 512                 # max token chunk (psum sizing)
NT = len(CHUNKS)
EJ = E // 128            # 5 e-tiles
CJ = CR // 128           # 6 cross-dim chunks
SCALE = float(1.0 / np.sqrt(D))
KP = 768                 # Q contraction padded to 3 DoubleRow pairs
ND = KP // 256           # DR pair count
WQSCALE = 64.0           # fp8 range scaling for Wq, undone in the bias copy


def _frags():
    fr = []
    for h in range(H):
        e0, e1 = D * h, D * h + D
        for j in range(e0 // 128, (e1 - 1) // 128 + 1):
            p0, p1 = max(0, e0 - 128 * j), min(128, e1 - 128 * j)
            fr.append((h, j, p0, p1))
    return fr


FRAGS = _frags()


def _emit(nc, tc, dr):
    import contextlib

    ctx = contextlib.ExitStack()
    with ctx:
        cpool = ctx.enter_context(tc.tile_pool(name="const", bufs=1))
        pq = ctx.enter_context(tc.tile_pool(name="pq", bufs=2, space="PSUM"))
        psc = ctx.enter_context(tc.tile_pool(name="psc", bufs=2, space="PSUM"))
        pav = ctx.enter_context(tc.tile_pool(name="pav", bufs=2, space="PSUM"))
        pop = ctx.enter_context(tc.tile_pool(name="pop", bufs=2, space="PSUM"))
        xpool = ctx.enter_context(tc.tile_pool(name="xpool", bufs=3))
        qpool = ctx.enter_context(tc.tile_pool(name="qpool", bufs=3))
        apool = ctx.enter_context(tc.tile_pool(name="apool", bufs=3))
        upool = ctx.enter_context(tc.tile_pool(name="upool", bufs=3))
        rpool = ctx.enter_context(tc.tile_pool(name="rpool", bufs=2))
        atpool = ctx.enter_context(tc.tile_pool(name="atpool", bufs=3))
        opool = ctx.enter_context(tc.tile_pool(name="opool", bufs=8))

        # ---- load constants/weights ----
        def load(name, shape, src, dt=F32R):
            t = cpool.tile(shape, dt, tag=name, name=name)
            nc.sync.dma_start(t[:], src)
            return t

        # load order matters: it sets DMA queue order, which gates when PE
        # can start. wq + chunk-0 x first (Q-proj starts early), wo last.
        def loadw(name, src2, nblk, cols, dt=BF16):
            t = cpool.tile([128, nblk, cols], dt, tag=name, name=name)
            nc.sync.dma_start(
                t[:], src2.rearrange("(b p) c -> p b c", p=128))
            return t

        # interleaved so Qproj's first matmul (needs wq0+x0 only) starts
        # after ~2 DMAs rather than after the whole wq+x chunk-0 load
        tw0 = CHUNKS[0][1]
        # Q-proj runs as fp8e4m3 DoubleRow: wq8/xt8 hold two 128-row
        # k-blocks side by side in the free dim, one matmul contracts 256
        wq8 = []
        xp0 = []
        for d in range(ND):
            w = cpool.tile([128, 2, E], FP8, tag=f"wq8_{d}", name=f"wq8_{d}")
            nc.sync.dma_start(
                w[:],
                dr["wq8"][256 * d:256 * (d + 1), :].rearrange(
                    "(two p) c -> p two c", two=2))
            wq8.append(w)
            xt = xpool.tile([128, 2, TC], FP8, tag=f"xp{d}", name=f"xp0_{d}")
            nc.sync.dma_start(
                xt[:, :, 0:tw0],
                dr["xt8"][256 * d:256 * (d + 1), 0:tw0].rearrange(
                    "(two p) t -> p two t", two=2))
            xp0.append(xt)
        bqt = load("bqt", [128, EJ], dr["bqt"][:], F32)
        ytp_t = loadw("yt", dr["yt"][:], CJ, S)
        wk_t = loadw("wk", dr["wk"][:], CJ, E)
        bkt = load("bkt", [128, EJ], dr["bkt"][:], F32)
        kmask = load("kmask", [128, len(FRAGS)], dr["kmask"][:], F32)
        ecols = load("ecols", [S, H * H], dr["ecols"][:], BF16)
        sel5 = load("sel5", [H, 128 * EJ], dr["sel5"][:], BF16)
        wv_t = loadw("wv", dr["wv"][:], CJ, E)
        bvr = load("bvr", [1, E], dr["bvr"][:], BF16)
        ones77 = load("ones77", [1, S], dr["ones77"][:], BF16)
        vmask = load("vmask", [S, 128 * len(FRAGS)], dr["vmask"][:], F32)
        wo_t = loadw("wo", dr["wo"][:], EJ, E)
        bot = load("bot", [128, EJ], dr["bot"][:], F32)
        wk_sb = [wk_t[:, c, :] for c in range(CJ)]
        wv_sb = [wv_t[:, c, :] for c in range(CJ)]
        wo_sb = [wo_t[:, c, :] for c in range(EJ)]
        ytp = [ytp_t[:, c, :] for c in range(CJ)]

        def emit_qproj(xp, tw):
            qs = []
            for j in range(EJ):
                qp = pq.tile([128, TC], F32, tag="qp", name=f"qp{j}")
                for d in range(ND):
                    nc.tensor.matmul(qp[0:128, 0:tw],
                                     wq8[d][:, :, 128 * j:128 * (j + 1)],
                                     xp[d][:, :, 0:tw],
                                     start=(d == 0), stop=(d == ND - 1),
                                     perf_mode=DR)
                # bias-add + fp8 range-scale undo, fused with the
                # PSUM->SBUF move on the scalar engine
                q = qpool.tile([128, TC], BF16, tag=f"q{j}", name=f"q{j}")
                nc.scalar.activation(q[0:128, 0:tw], qp[0:128, 0:tw],
                                     AF.Identity, scale=1.0 / WQSCALE,
                                     bias=bqt[:, j:j + 1])
                qs.append(q)
            return qs

        # chunk-0 Q-projection first: PE is in-order, so this must precede
        # the K/V setup in program order to start as soon as wq+x arrive
        qs0 = emit_qproj(xp0, tw0)

        # ---- K projection -> zero-padded per-fragment staging tiles ----
        # kstage[fi] = (k'_tile + bk) * mask_fi  (mask zeroes rows outside
        # the head fragment; done full-partition because engine ops need
        # 32-aligned start partitions)
        kstage = [cpool.tile([128, S], BF16, tag=f"ks{fi}", name=f"ks{fi}")
                  for fi in range(len(FRAGS))]
        for j in range(EJ):
            kp = pq.tile([128, S], F32, tag="qp")
            for c in range(CJ):
                nc.tensor.matmul(kp[:], wk_sb[c][:, 128 * j:128 * (j + 1)],
                                 ytp[c], start=(c == 0), stop=(c == CJ - 1))
            for fi, (h, jj, p0, p1) in enumerate(FRAGS):
                if jj != j:
                    continue
                nc.vector.tensor_scalar(kstage[fi][:], kp[:],
                                        bkt[:, j:j + 1], kmask[:, fi:fi + 1],
                                        mybir.AluOpType.add,
                                        mybir.AluOpType.mult)

        # V projection is emitted inside iteration 0, after chunk-0's
        # scores: its vb tiles are first consumed by the pipelined finish
        # in iteration 1, so this keeps the PE off the wv DMA critical path
        vb = [cpool.tile([S, 128], BF16, tag=f"vb{fi}", name=f"vb{fi}")
              for fi in range(len(FRAGS))]

        def emit_vproj():
            for (n0, n1) in ((0, 512), (512, E)):
                vp = psc.tile([S, n1 - n0], F32, tag="sc")
                for c in range(CJ):
                    nc.tensor.matmul(vp[:], ytp[c], wv_sb[c][:, n0:n1],
                                     start=(c == 0), stop=False)
                nc.tensor.matmul(vp[:], ones77[:], bvr[:, n0:n1],
                                 start=False, stop=True)
                for fi, (h, j, p0, p1) in enumerate(FRAGS):
                    c0 = 128 * j
                    if not (n0 <= c0 and c0 + 128 <= n1):
                        continue
                    nc.vector.tensor_mul(vb[fi][:],
                                         vp[:, c0 - n0:c0 - n0 + 128],
                                         vmask[:, 128 * fi:128 * (fi + 1)])

        emit_vproj()

        # ---- main loop over token chunks (software-pipelined) ----
        # The normalization (bc sel-matmuls + at multiplies) and the
        # O-projection of chunk i are emitted during chunk i+1, after its
        # Q-projection: the 3.3us DVE reciprocal of chunk i then overlaps
        # chunk i+1's Q matmuls instead of stalling the PE at bc.
        def emit_finish(st):
            aps, rec8, t0, tw = st
            us = []
            for j in range(EJ):
                av = pav.tile([128, TC], F32, tag="av")
                frs = [(fi, f) for fi, f in enumerate(FRAGS) if f[1] == j]
                for i, (fi, (h, jj, p0, p1)) in enumerate(frs):
                    nc.tensor.matmul(av[0:128, 0:tw], vb[fi][:],
                                     aps[h][0:S, 0:tw],
                                     start=(i == 0), stop=(i == len(frs) - 1))
                u = upool.tile([128, TC], BF16, tag=f"u{j}")
                nc.vector.tensor_scalar_add(u[0:128, 0:tw], av[0:128, 0:tw],
                                            0.0)
                us.append(u)
            attn = []
            for j in range(EJ):
                bc = pav.tile([128, TC], F32, tag="av")
                nc.tensor.matmul(bc[0:128, 0:tw],
                                 sel5[:, 128 * j:128 * (j + 1)],
                                 rec8[0:H, 0:tw])
                at = atpool.tile([128, TC], BF16, tag=f"at{j}")
                nc.vector.tensor_mul(at[0:128, 0:tw], us[j][0:128, 0:tw],
                                     bc[0:128, 0:tw])
                attn.append(at)
            for p in range(EJ):
                op = pop.tile([128, TC], F32, tag="op")
                for j in range(EJ):
                    nc.tensor.matmul(op[0:128, 0:tw],
                                     wo_sb[j][:, 128 * p:128 * (p + 1)],
                                     attn[j][0:128, 0:tw],
                                     start=(j == 0), stop=(j == EJ - 1))
                ob = opool.tile([128, TC], BF16, tag="ob")
                nc.scalar.activation(ob[0:128, 0:tw], op[0:128, 0:tw],
                                     AF.Identity, bias=bot[:, p:p + 1])
                nc.sync.dma_start(dr["ot"][128 * p:128 * (p + 1), t0:t0 + tw],
                                  ob[0:128, 0:tw])

        def fetch_x(it):
            t0, tw = CHUNKS[it]
            xp = []
            for d in range(ND):
                xt = xpool.tile([128, 2, TC], FP8, tag=f"xp{d}")
                nc.sync.dma_start(
                    xt[:, :, 0:tw],
                    dr["xt8"][256 * d:256 * (d + 1), t0:t0 + tw].rearrange(
                        "(two p) t -> p two t", two=2))
                xp.append(xt)
            return xp

        pending = None
        xp = xp0
        for it in range(NT):
            t0, tw = CHUNKS[it]

            # Q projection
            qs = qs0 if it == 0 else emit_qproj(xp, tw)

            # prefetch next chunk's x now so its Q-proj never waits on DMA
            if it + 1 < NT:
                xp = fetch_x(it + 1)

            # finish the previous chunk (its reciprocal ran during Qproj)
            if pending is not None:
                emit_finish(pending)

            # scores -> exp; sumexp accumulated in one [H, TC] psum tile.
            # The s8 matmul for head h-1 is issued between the scores
            # matmuls of heads h and h+1 so the in-order PE never waits
            # directly on the exp (scalar) it consumes.
            s8 = pop.tile([H, TC], F32, tag="op", name="s8")
            aps = []

            def emit_s8(h):
                nc.tensor.matmul(s8[0:H, 0:tw], ecols[:, H * h:H * h + H],
                                 aps[h][0:S, 0:tw],
                                 start=(h == 0), stop=(h == H - 1),
                                 skip_group_check=True)

            for h in range(H):
                frs = [(fi, f) for fi, f in enumerate(FRAGS) if f[0] == h]
                sc = psc.tile([S, TC], F32, tag="sc")
                for i, (fi, (hh, j, p0, p1)) in enumerate(frs):
                    nc.tensor.matmul(sc[0:S, 0:tw], kstage[fi][:],
                                     qs[j][0:128, 0:tw],
                                     start=(i == 0), stop=(i == len(frs) - 1))
                if h > 0:
                    emit_s8(h - 1)
                a = apool.tile([S, TC], BF16, tag=f"a{h}")
                nc.scalar.activation(a[0:S, 0:tw], sc[0:S, 0:tw], AF.Exp,
                                     scale=SCALE)
                aps.append(a)
            emit_s8(H - 1)

            rec8 = rpool.tile([H, TC], BF16, tag="rec8")
            with nc.allow_low_precision(reason="recip rounded to f32r"):
                nc.vector.reciprocal(rec8[0:H, 0:tw], s8[0:H, 0:tw])

            pending = (aps, rec8, t0, tw)

        emit_finish(pending)


def build_program():
    nc = bacc.Bacc("TRN2", target_bir_lowering=False, debug=False, num_devices=B)
    dr = {}

    def din(name, shape, dt=F32):
        dr[name] = nc.dram_tensor(name, shape, dt, kind="ExternalInput")
        return dr[name]

    din("xt8", [KP, T], FP8)
    din("yt", [CR, S], BF16)
    din("wq8", [KP, E], FP8)
    din("wk", [CR, E], BF16)
    din("wv", [CR, E], BF16)
    din("wo", [E, E], BF16)
    din("bqt", [128, EJ])
    din("bkt", [128, EJ])
    din("bot", [128, EJ])
    din("bvr", [1, E], BF16)
    din("ecols", [S, H * H], BF16)
    din("ones77", [1, S], BF16)
    din("kmask", [128, len(FRAGS)])
    din("vmask", [S, 128 * len(FRAGS)])
    din("sel5", [H, 128 * EJ], BF16)
    dr["ot"] = nc.dram_tensor("ot", [E, T], BF16, kind="ExternalOutput")

    with tile.TileContext(nc) as tc:
        _emit(nc, tc, {k: v[:] for k, v in dr.items()})
    nc.compile()
    return nc


def make_in_maps(x, y, Wq, bq, Wk, bk, Wv, bv, Wo, bo):
    import ml_dtypes
    bf16 = ml_dtypes.bfloat16
    fp8 = mybir.dt.np(FP8)
    f = lambda a: np.ascontiguousarray(np.asarray(a, dtype=np.float32))
    fb = lambda a: np.ascontiguousarray(
        np.asarray(a, dtype=np.float32).astype(bf16))
    ecols = np.zeros((S, H * H), bf16)
    for h in range(H):
        ecols[:, H * h + h] = 1.0
    kmask = np.zeros((128, len(FRAGS)), np.float32)
    vmask = np.zeros((S, 128 * len(FRAGS)), np.float32)
    for fi, (h, j, p0, p1) in enumerate(FRAGS):
        kmask[p0:p1, fi] = 1.0
        vmask[:, 128 * fi + p0:128 * fi + p1] = 1.0
    sel5 = np.zeros((H, 128 * EJ), bf16)
    for e in range(E):
        sel5[e // D, e] = 1.0
    wq8 = np.zeros((KP, E), np.float32)
    wq8[:E] = np.asarray(Wq, np.float32) * WQSCALE
    shared = dict(
        wq8=np.ascontiguousarray(wq8.astype(fp8)), wk=fb(Wk), wv=fb(Wv),
        wo=fb(Wo),
        bqt=f(np.asarray(bq, np.float32).reshape(EJ, 128).T),
        bkt=f(np.asarray(bk, np.float32).reshape(EJ, 128).T),
        bot=f(np.asarray(bo, np.float32).reshape(EJ, 128).T),
        bvr=fb(np.asarray(bv, np.float32).reshape(1, E)),
        ecols=ecols,
        ones77=np.ones((1, S), bf16),
        kmask=kmask,
        vmask=vmask,
        sel5=sel5,
    )
    x = np.asarray(x, np.float32)
    y = np.asarray(y, np.float32)
    in_maps = []
    for b in range(B):
        m = dict(shared)
        xt8 = np.zeros((KP, T), np.float32)
        xt8[:E] = x[b].T
        m["xt8"] = np.ascontiguousarray(xt8.astype(fp8))
        m["yt"] = fb(y[b].T)
        in_maps.append(m)
    return in_maps


def assemble_output(results):
    return np.stack([results[b]["ot"].T.astype(np.float32) for b in range(B)], axis=0)


_PROG = None


def _prog():
    global _PROG
    if _PROG is None:
        _PROG = build_program()
    return _PROG


def kernel(x, y, Wq, bq, Wk, bk, Wv, bv, Wo, bo):
    nc = _prog()
    in_maps = make_in_maps(x, y, Wq, bq, Wk, bk, Wv, bv, Wo, bo)
    res = run_bass_kernel_spmd(nc, in_maps, core_ids=list(range(B)))
    return assemble_output(res.results)


# revision 28
# speedup vs baseline: 1.0169x; 1.0169x over previous
"""Cross-attention kernel for Trainium2 (8 NeuronCores, batch-parallel).

Reference computation (per batch element b):
    q = x @ Wq + bq            # [T, E]
    k = y @ Wk + bk            # [S, E]
    v = y @ Wv + bv            # [S, E]
    per head h (D=80): scores = q_h @ k_h.T / sqrt(D); A = softmax(scores)
    attn = concat_h(A @ v_h)   # [T, E]
    out = attn @ Wo + bo       # [T, E]

Sharding: batch (8) across the 8 cores, one batch element per core.

On-chip layout is feature-major (x and the output are transposed on the
host so every DMA is a contiguous row load/store and no on-chip
transposes are needed):
    xt  = x[b].T   [E, T]   -> X' tiles [128 c, 512 t] (bf16)
    q'  = Wq.T@X'  [E, T]   feature-major (e on partitions)
    k'  staged per head fragment with zero padding (80-dim heads vs
        128-partition tiles; matmul operand base partitions must be 0)
    U   = exp(scores) @ V computed unnormalized; the softmax denominator
        r8[h,t] = sum_s exp(scores) is accumulated with a ones-matmul
        (ecols) into one [H, TC] psum tile, inverted on DVE, and
        broadcast onto the 5 attn e'-tiles with 5 tiny sel-matmuls
        (K=8); normalization is one DVE multiply per tile fused with
        the PSUM->SBUF move (out in bf16 for the O-proj).
    out' = Wo.T @ attn' [E, T]; host transposes back.

HBM-resident operands (x, y, Wq, Wk, Wv, Wo) are bf16: same PE rate as
float32r at free-dim 512, half the DMA traffic, and full rate for the
free-dim-77 K-projection (f32r would take a 4x penalty there).
"""

import numpy as np

import concourse.bass as bass
import concourse.mybir as mybir
import concourse.tile as tile
from concourse import bacc
from concourse.bass_utils import run_bass_kernel_spmd

F32 = mybir.dt.float32
F32R = mybir.dt.float32r
BF16 = mybir.dt.bfloat16
FP8 = mybir.dt.float8e4
DR = mybir.MatmulPerfMode.DoubleRow
AF = mybir.ActivationFunctionType

B, T, E, CR, H, D, S = 8, 4096, 640, 768, 8, 80, 77
CHUNKS = ([(0, 256), (256, 256)]
          + [(512 + 512 * i, 512) for i in range(6)]
          + [(3584, 256), (3840, 256)])
TC = 512                 # max token chunk (psum sizing)
NT = len(CHUNKS)
EJ = E // 128            # 5 e-tiles
CJ = CR // 128           # 6 cross-dim chunks
SCALE = float(1.0 / np.sqrt(D))
KP = 768                 # Q contraction padded to 3 DoubleRow pairs
ND = KP // 256           # DR pair count
WQSCALE = 64.0           # fp8 range scaling for Wq, undone in the bias copy


def _frags():
    fr = []
    for h in range(H):
        e0, e1 = D * h, D * h + D
        for j in range(e0 // 128, (e1 - 1) // 128 + 1):
            p0, p1 = max(0, e0 - 128 * j), min(128, e1 - 128 * j)
            fr.append((h, j, p0, p1))
    return fr


FRAGS = _frags()


def _emit(nc, tc, dr):
    import contextlib

    ctx = contextlib.ExitStack()
    with ctx:
        cpool = ctx.enter_context(tc.tile_pool(name="const", bufs=1))
        pq = ctx.enter_context(tc.tile_pool(name="pq", bufs=2, space="PSUM"))
        psc = ctx.enter_context(tc.tile_pool(name="psc", bufs=2, space="PSUM"))
        pav = ctx.enter_context(tc.tile_pool(name="pav", bufs=2, space="PSUM"))
        pop = ctx.enter_context(tc.tile_pool(name="pop", bufs=2, space="PSUM"))
        xpool = ctx.enter_context(tc.tile_pool(name="xpool", bufs=2))
        qpool = ctx.enter_context(tc.tile_pool(name="qpool", bufs=2))
        apool = ctx.enter_context(tc.tile_pool(name="apool", bufs=2))
        upool = ctx.enter_context(tc.tile_pool(name="upool", bufs=2))
        rpool = ctx.enter_context(tc.tile_pool(name="rpool", bufs=2))
        atpool = ctx.enter_context(tc.tile_pool(name="atpool", bufs=2))
        opool = ctx.enter_context(tc.tile_pool(name="opool", bufs=6))

        # ---- load constants/weights ----
        def load(name, shape, src, dt=F32R):
            t = cpool.tile(shape, dt, tag=name, name=name)
            nc.sync.dma_start(t[:], src)
            return t

        # load order matters: it sets DMA queue order, which gates when PE
        # can start. wq + chunk-0 x first (Q-proj starts early), wo last.
        def loadw(name, src2, nblk, cols, dt=BF16):
            t = cpool.tile([128, nblk, cols], dt, tag=name, name=name)
            nc.sync.dma_start(
                t[:], src2.rearrange("(b p) c -> p b c", p=128))
            return t

        # interleaved so Qproj's first matmul (needs wq0+x0 only) starts
        # after ~2 DMAs rather than after the whole wq+x chunk-0 load
        tw0 = CHUNKS[0][1]
        # Q-proj runs as fp8e4m3 DoubleRow: wq8/xt8 hold two 128-row
        # k-blocks side by side in the free dim, one matmul contracts 256
        wq8 = []
        xp0 = []
        for d in range(ND):
            w = cpool.tile([128, 2, E], FP8, tag=f"wq8_{d}", name=f"wq8_{d}")
            nc.sync.dma_start(
                w[:],
                dr["wq8"][256 * d:256 * (d + 1), :].rearrange(
                    "(two p) c -> p two c", two=2))
            wq8.append(w)
            xt = xpool.tile([128, 2, TC], FP8, tag=f"xp{d}", name=f"xp0_{d}")
            nc.sync.dma_start(
                xt[:, :, 0:tw0],
                dr["xt8"][256 * d:256 * (d + 1), 0:tw0].rearrange(
                    "(two p) t -> p two t", two=2))
            xp0.append(xt)
        bqt = load("bqt", [128, EJ], dr["bqt"][:], F32)
        ytp_t = loadw("yt", dr["yt"][:], CJ, S)
        wk_t = loadw("wk", dr["wk"][:], CJ, E)
        bkt = load("bkt", [128, EJ], dr["bkt"][:], F32)
        kmask = load("kmask", [128, len(FRAGS)], dr["kmask"][:], F32)
        ecols = load("ecols", [S, H * H], dr["ecols"][:], BF16)
        sel5 = load("sel5", [H, 128 * EJ], dr["sel5"][:], BF16)
        wv_t = loadw("wv", dr["wv"][:], CJ, E)
        bvr = load("bvr", [1, E], dr["bvr"][:], BF16)
        ones77 = load("ones77", [1, S], dr["ones77"][:], BF16)
        vmask = load("vmask", [S, 128 * len(FRAGS)], dr["vmask"][:], F32)
        wo_t = loadw("wo", dr["wo"][:], EJ, E)
        bot = load("bot", [128, EJ], dr["bot"][:], F32)
        wk_sb = [wk_t[:, c, :] for c in range(CJ)]
        wv_sb = [wv_t[:, c, :] for c in range(CJ)]
        wo_sb = [wo_t[:, c, :] for c in range(EJ)]
        ytp = [ytp_t[:, c, :] for c in range(CJ)]

        def emit_qproj(xp, tw):
            qs = []
            for j in range(EJ):
                qp = pq.tile([128, TC], F32, tag="qp", name=f"qp{j}")
                for d in range(ND):
                    nc.tensor.matmul(qp[0:128, 0:tw],
                                     wq8[d][:, :, 128 * j:128 * (j + 1)],
                                     xp[d][:, :, 0:tw],
                                     start=(d == 0), stop=(d == ND - 1),
                                     perf_mode=DR)
                # bias-add + fp8 range-scale undo, fused with the
                # PSUM->SBUF move on the scalar engine
                q = qpool.tile([128, TC], BF16, tag=f"q{j}", name=f"q{j}")
                nc.scalar.activation(q[0:128, 0:tw], qp[0:128, 0:tw],
                                     AF.Identity, scale=1.0 / WQSCALE,
                                     bias=bqt[:, j:j + 1])
                qs.append(q)
            return qs

        # chunk-0 Q-projection first: PE is in-order, so this must precede
        # the K/V setup in program order to start as soon as wq+x arrive
        qs0 = emit_qproj(xp0, tw0)

        # ---- K projection -> zero-padded per-fragment staging tiles ----
        # kstage[fi] = (k'_tile + bk) * mask_fi  (mask zeroes rows outside
        # the head fragment; done full-partition because engine ops need
        # 32-aligned start partitions)
        kstage = [cpool.tile([128, S], BF16, tag=f"ks{fi}", name=f"ks{fi}")
                  for fi in range(len(FRAGS))]
        for j in range(EJ):
            kp = pq.tile([128, S], F32, tag="qp")
            for c in range(CJ):
                nc.tensor.matmul(kp[:], wk_sb[c][:, 128 * j:128 * (j + 1)],
                                 ytp[c], start=(c == 0), stop=(c == CJ - 1))
            for fi, (h, jj, p0, p1) in enumerate(FRAGS):
                if jj != j:
                    continue
                nc.vector.tensor_scalar(kstage[fi][:], kp[:],
                                        bkt[:, j:j + 1], kmask[:, fi:fi + 1],
                                        mybir.AluOpType.add,
                                        mybir.AluOpType.mult)

        # V projection is emitted inside iteration 0, after chunk-0's
        # scores: its vb tiles are first consumed by the pipelined finish
        # in iteration 1, so this keeps the PE off the wv DMA critical path
        vb = [cpool.tile([S, 128], BF16, tag=f"vb{fi}", name=f"vb{fi}")
              for fi in range(len(FRAGS))]

        def emit_vproj():
            for (n0, n1) in ((0, 512), (512, E)):
                vp = psc.tile([S, n1 - n0], F32, tag="sc")
                for c in range(CJ):
                    nc.tensor.matmul(vp[:], ytp[c], wv_sb[c][:, n0:n1],
                                     start=(c == 0), stop=False)
                nc.tensor.matmul(vp[:], ones77[:], bvr[:, n0:n1],
                                 start=False, stop=True)
                for fi, (h, j, p0, p1) in enumerate(FRAGS):
                    c0 = 128 * j
                    if not (n0 <= c0 and c0 + 128 <= n1):
                        continue
                    nc.vector.tensor_mul(vb[fi][:],
                                         vp[:, c0 - n0:c0 - n0 + 128],
                                         vmask[:, 128 * fi:128 * (fi + 1)])

        emit_vproj()

        # ---- main loop over token chunks (software-pipelined) ----
        # The normalization (bc sel-matmuls + at multiplies) and the
        # O-projection of chunk i are emitted during chunk i+1, after its
        # Q-projection: the 3.3us DVE reciprocal of chunk i then overlaps
        # chunk i+1's Q matmuls instead of stalling the PE at bc.
        def emit_finish(st):
            aps, rec8, t0, tw = st
            us = []
            for j in range(EJ):
                av = pav.tile([128, TC], F32, tag="av")
                frs = [(fi, f) for fi, f in enumerate(FRAGS) if f[1] == j]
                for i, (fi, (h, jj, p0, p1)) in enumerate(frs):
                    nc.tensor.matmul(av[0:128, 0:tw], vb[fi][:],
                                     aps[h][0:S, 0:tw],
                                     start=(i == 0), stop=(i == len(frs) - 1))
                u = upool.tile([128, TC], BF16, tag=f"u{j}")
                nc.vector.tensor_scalar_add(u[0:128, 0:tw], av[0:128, 0:tw],
                                            0.0)
                us.append(u)
            attn = []
            for j in range(EJ):
                bc = pav.tile([128, TC], F32, tag="av")
                nc.tensor.matmul(bc[0:128, 0:tw],
                                 sel5[:, 128 * j:128 * (j + 1)],
                                 rec8[0:H, 0:tw])
                at = atpool.tile([128, TC], BF16, tag=f"at{j}")
                nc.vector.tensor_mul(at[0:128, 0:tw], us[j][0:128, 0:tw],
                                     bc[0:128, 0:tw])
                attn.append(at)
            for p in range(EJ):
                if drain[0] and p % 2 == 0:
                    op = pq.tile([128, TC], F32, tag="qp", name=f"opq{p}")
                else:
                    op = pop.tile([128, TC], F32, tag="op")
                for j in range(EJ):
                    nc.tensor.matmul(op[0:128, 0:tw],
                                     wo_sb[j][:, 128 * p:128 * (p + 1)],
                                     attn[j][0:128, 0:tw],
                                     start=(j == 0), stop=(j == EJ - 1))
                ob = opool.tile([128, TC], BF16, tag="ob")
                nc.scalar.activation(ob[0:128, 0:tw], op[0:128, 0:tw],
                                     AF.Identity, bias=bot[:, p:p + 1])
                nc.sync.dma_start(dr["ot"][128 * p:128 * (p + 1), t0:t0 + tw],
                                  ob[0:128, 0:tw])

        def fetch_x(it):
            t0, tw = CHUNKS[it]
            xp = []
            for d in range(ND):
                xt = xpool.tile([128, 2, TC], FP8, tag=f"xp{d}")
                nc.sync.dma_start(
                    xt[:, :, 0:tw],
                    dr["xt8"][256 * d:256 * (d + 1), t0:t0 + tw].rearrange(
                        "(two p) t -> p two t", two=2))
                xp.append(xt)
            return xp

        pending = None
        drain = [False]
        xp = xp0
        for it in range(NT):
            t0, tw = CHUNKS[it]

            # prefetch next chunk's x first so its DMA starts earliest
            xp_next = fetch_x(it + 1) if it + 1 < NT else None

            # Q projection
            qs = qs0 if it == 0 else emit_qproj(xp, tw)
            xp = xp_next

            # finish the previous chunk (its reciprocal ran during Qproj)
            if pending is not None:
                emit_finish(pending)

            # scores -> exp; sumexp accumulated in one [H, TC] psum tile.
            # The s8 matmul for head h-1 is issued between the scores
            # matmuls of heads h and h+1 so the in-order PE never waits
            # directly on the exp (scalar) it consumes.
            s8 = pop.tile([H, TC], F32, tag="op", name="s8")
            aps = []

            def emit_s8(h):
                nc.tensor.matmul(s8[0:H, 0:tw], ecols[:, H * h:H * h + H],
                                 aps[h][0:S, 0:tw],
                                 start=(h == 0), stop=(h == H - 1),
                                 skip_group_check=True)

            for h in range(H):
                frs = [(fi, f) for fi, f in enumerate(FRAGS) if f[0] == h]
                sc = psc.tile([S, TC], F32, tag="sc")
                for i, (fi, (hh, j, p0, p1)) in enumerate(frs):
                    nc.tensor.matmul(sc[0:S, 0:tw], kstage[fi][:],
                                     qs[j][0:128, 0:tw],
                                     start=(i == 0), stop=(i == len(frs) - 1))
                if h > 0:
                    emit_s8(h - 1)
                a = apool.tile([S, TC], BF16, tag=f"a{h}")
                nc.scalar.activation(a[0:S, 0:tw], sc[0:S, 0:tw], AF.Exp,
                                     scale=SCALE)
                aps.append(a)
            emit_s8(H - 1)

            rec8 = rpool.tile([H, TC], BF16, tag="rec8")
            with nc.allow_low_precision(reason="recip rounded to f32r"):
                nc.vector.reciprocal(rec8[0:H, 0:tw], s8[0:H, 0:tw])

            pending = (aps, rec8, t0, tw)

        drain[0] = True
        emit_finish(pending)


def build_program():
    nc = bacc.Bacc("TRN2", target_bir_lowering=False, debug=False, num_devices=B)
    dr = {}

    def din(name, shape, dt=F32):
        dr[name] = nc.dram_tensor(name, shape, dt, kind="ExternalInput")
        return dr[name]

    din("xt8", [KP, T], FP8)
    din("yt", [CR, S], BF16)
    din("wq8", [KP, E], FP8)
    din("wk", [CR, E], BF16)
    din("wv", [CR, E], BF16)
    din("wo", [E, E], BF16)
    din("bqt", [128, EJ])
    din("bkt", [128, EJ])
    din("bot", [128, EJ])
    din("bvr", [1, E], BF16)
    din("ecols", [S, H * H], BF16)
    din("ones77", [1, S], BF16)
    din("kmask", [128, len(FRAGS)])
    din("vmask", [S, 128 * len(FRAGS)])
    din("sel5", [H, 128 * EJ], BF16)
    dr["ot"] = nc.dram_tensor("ot", [E, T], BF16, kind="ExternalOutput")

    with tile.TileContext(nc) as tc:
        _emit(nc, tc, {k: v[:] for k, v in dr.items()})
    nc.compile()
    return nc


def make_in_maps(x, y, Wq, bq, Wk, bk, Wv, bv, Wo, bo):
    import ml_dtypes
    bf16 = ml_dtypes.bfloat16
    fp8 = mybir.dt.np(FP8)
    f = lambda a: np.ascontiguousarray(np.asarray(a, dtype=np.float32))
    fb = lambda a: np.ascontiguousarray(
        np.asarray(a, dtype=np.float32).astype(bf16))
    ecols = np.zeros((S, H * H), bf16)
    for h in range(H):
        ecols[:, H * h + h] = 1.0
    kmask = np.zeros((128, len(FRAGS)), np.float32)
    vmask = np.zeros((S, 128 * len(FRAGS)), np.float32)
    for fi, (h, j, p0, p1) in enumerate(FRAGS):
        kmask[p0:p1, fi] = 1.0
        vmask[:, 128 * fi + p0:128 * fi + p1] = 1.0
    sel5 = np.zeros((H, 128 * EJ), bf16)
    for e in range(E):
        sel5[e // D, e] = 1.0
    wq8 = np.zeros((KP, E), np.float32)
    wq8[:E] = np.asarray(Wq, np.float32) * WQSCALE
    shared = dict(
        wq8=np.ascontiguousarray(wq8.astype(fp8)), wk=fb(Wk), wv=fb(Wv),
        wo=fb(Wo),
        bqt=f(np.asarray(bq, np.float32).reshape(EJ, 128).T),
        bkt=f(np.asarray(bk, np.float32).reshape(EJ, 128).T),
        bot=f(np.asarray(bo, np.float32).reshape(EJ, 128).T),
        bvr=fb(np.asarray(bv, np.float32).reshape(1, E)),
        ecols=ecols,
        ones77=np.ones((1, S), bf16),
        kmask=kmask,
        vmask=vmask,
        sel5=sel5,
    )
    x = np.asarray(x, np.float32)
    y = np.asarray(y, np.float32)
    in_maps = []
    for b in range(B):
        m = dict(shared)
        xt8 = np.zeros((KP, T), np.float32)
        xt8[:E] = x[b].T
        m["xt8"] = np.ascontiguousarray(xt8.astype(fp8))
        m["yt"] = fb(y[b].T)
        in_maps.append(m)
    return in_maps


def assemble_output(results):
    return np.stack([results[b]["ot"].T.astype(np.float32) for b in range(B)], axis=0)


_PROG = None


def _prog():
    global _PROG
    if _PROG is None:
        _PROG = build_program()
    return _PROG


def kernel(x, y, Wq, bq, Wk, bk, Wv, bv, Wo, bo):
    nc = _prog()
    in_maps = make_in_maps(x, y, Wq, bq, Wk, bk, Wv, bv, Wo, bo)
    res = run_bass_kernel_spmd(nc, in_maps, core_ids=list(range(B)))
    return assemble_output(res.results)
